# revision 1
# baseline (speedup 1.0000x reference)
"""Trainium2 Bass kernel for the masked-bottleneck + coord-attention block.

Sharding: data-parallel over batch (B=16 -> 8 cores x 2 samples), weights
replicated. Everything below runs per-core on its 2-sample shard.

Per-sample dataflow (channels on partitions, 512 = 4 chunks of 128):
  s_logits   : PE matmul with sm_w replicated across all 128 output columns,
               so the [1,HW] logit row lands replicated on all partitions.
  signmask   : ACT Sign(logits + b) -> {-1,0,1} replicated mask (row 0 is the
               mask row). Dilated mask built on a [58,58] 2D view via tiny
               TT-max ops + partition-shift DMAs, then GPSIMD
               partition_broadcast back to [128,HW].
  conv1      : PE; epilogue ACT Relu(cm1*z + cm1*b1) then GPSIMD
               (sdil max 0) * h fused mask-multiply with free pooled sums.
  conv2      : 9 shifted-AP matmuls over the 58x58 zero-padded h1m.
  conv3+CA   : pools of h3 are derived by linearity (W3 @ pool(h2m) + b3*pool(s)),
               so h3 is never materialized: conv3 psum -> DVE *ah*cm3 -> DVE *aw
               -> PE accumulates identity@x (residual) -> ACT Relu -> out.
  channel masks: is_gt on tiny matmul outputs; pooled inputs come from ACT
               accum_out / STT accum_out side-channels.
"""

import os
import sys

for _p in ("/opt/trn_rl_repo", os.path.expanduser("~/.axon_site/_ro/trn_rl_repo")):
    if os.path.isdir(_p) and _p not in sys.path:
        sys.path.insert(0, _p)

import numpy as np
from contextlib import ExitStack

import concourse.bass as bass
from concourse import bacc
import concourse.mybir as mybir
import concourse.tile as tile
from concourse import library_config
from concourse.bass_utils import run_bass_kernel_spmd

f32 = mybir.dt.float32
AF = mybir.ActivationFunctionType
OP = mybir.AluOpType
AX = mybir.AxisListType

NCORES = 8
BS = 2                  # samples per core
KC = 4                  # 512 input channels -> 4 chunks of 128
MC = 4                  # 512 output channels -> 4 chunks of 128
WID = 128
MIP = 16
H = W = 56
NPIX = H * W            # 3136
PW = 58                 # zero-padded 2D side
PADN = PW * PW          # 3364
TW = 448                # slog/conv1/conv2 N-tile (8 rows)
NT = NPIX // TW         # 7
RT = TW // W            # 8 rows per tile
# stage I (conv3/CA/residual) tiling: 784 cols = 2 halves of 392 (7 rows each)
IW = 784
NIT = NPIX // IW        # 4
IH = 392                # half width (7 rows)
IR = 7                  # rows per half


KSTAGE = int(os.environ.get("KSTAGE", "9"))
KREPEAT = int(os.environ.get("KREPEAT", "1"))


def _emit(nc, tc, ctx, d):
    sing = ctx.enter_context(tc.tile_pool(name="sing", bufs=1))
    xcp = ctx.enter_context(tc.tile_pool(name="xcp", bufs=4))
    big1 = ctx.enter_context(tc.tile_pool(name="big1", bufs=1))
    mid = ctx.enter_context(tc.tile_pool(name="mid", bufs=2))
    outp = ctx.enter_context(tc.tile_pool(name="outp", bufs=2))
    sm2 = ctx.enter_context(tc.tile_pool(name="sm2", bufs=2))
    dil1 = ctx.enter_context(tc.tile_pool(name="dil1", bufs=1))
    rows1 = ctx.enter_context(tc.tile_pool(name="rows1", bufs=1))
    pbig = ctx.enter_context(tc.tile_pool(name="pbig", bufs=2, space="PSUM"))
    pc3 = ctx.enter_context(tc.tile_pool(name="pc3", bufs=2, space="PSUM"))
    pvec = ctx.enter_context(tc.tile_pool(name="pvec", bufs=2, space="PSUM"))

    nc.gpsimd.load_library(library_config.mlp)

    # ---- weights / constants (loaded once) ----
    def wload(name, shape):
        t = sing.tile(shape, f32, name=name)
        nc.sync.dma_start(out=t, in_=d[name].ap())
        return t

    w1T = wload("w1T", [128, KC, 128])
    w2T = wload("w2T", [128, 9, 128])
    w3T = wload("w3T", [128, MC, 128])
    smw = wload("smw", [128, KC, 128])
    cm1w = wload("cm1w", [128, KC, 128])
    cm2w = wload("cm2w", [128, 128])
    cm3w = wload("cm3w", [128, MC, 128])
    caw1 = wload("caw1", [128, KC, MIP])
    cawh = wload("cawh", [MIP, MC, 128])
    caww = wload("caww", [MIP, MC, 128])
    idn = wload("idn", [128, 128])
    b1v = wload("b1v", [128, 1])
    b2v = wload("b2v", [128, 1])
    b3r = wload("b3r", [1, 512])
    smbneg = wload("smbneg", [128, 1])
    cm1nb = wload("cm1nb", [128, 1])
    cm2nb = wload("cm2nb", [128, 1])
    cm3nb = wload("cm3nb", [128, MC])
    cab1p3 = wload("cab1p3", [MIP, 1])
    cabh = wload("cabh", [128, MC])
    cabw = wload("cabw", [128, MC])
    ones58 = sing.tile([PW, 1], f32)
    nc.vector.memset(ones58, 1.0)

    x_d = d["x"]
    out_d = d["out"]

    for rep in range(KREPEAT):
      for s in range(BS):
        # ---------------- stage A: load x ----------------
        xk = []
        for k in range(KC):
            xt = xcp.tile([128, NPIX], f32, name=f"x_s{s}k{k}", tag="xc")
            nc.sync.dma_start(
                out=xt, in_=x_d[s, 128 * k:128 * (k + 1), :, :].rearrange("c h w -> c (h w)"))
            xk.append(xt)

        # ---------------- stage B: spatial-mask logits (replicated) ----------
        signmask = big1.tile([128, NPIX], f32, name=f"signmask{s}", tag="signmask")
        for t in range(NT):
            ps = pbig.tile([128, TW], f32, name=f"ps_slog{s}_{t}", tag="pbig")
            for k in range(KC):
                nc.tensor.matmul(ps[:, :], smw[:, k, :], xk[k][:, TW * t:TW * (t + 1)],
                                 start=(k == 0), stop=(k == KC - 1))
            nc.vector.tensor_scalar(out=signmask[:, TW * t:TW * (t + 1)], in0=ps[:, :],
                                    scalar1=smbneg[:, :], scalar2=None, op0=OP.is_gt)

        if KSTAGE < 2:
            nc.sync.dma_start(out=out_d[s, 0:128, :, :].rearrange("c h w -> c (h w)"),
                              in_=signmask[:, :])
            continue
        # ---------------- stage C: cm1 from pooled x ----------------
        px = sm2.tile([128, KC], f32, name=f"px{s}", tag="px")
        h2m = big1.tile([128, NPIX], f32, name=f"h2m{s}", tag="h2m")  # also px scratch
        for k in range(KC):
            nc.scalar.activation(out=h2m[:, :], in_=xk[k][:, :], func=AF.Copy,
                                 accum_out=px[:, k:k + 1])
        pl1 = pvec.tile([128, 1], f32, name=f"pl1{s}", tag="pvec")
        for k in range(KC):
            nc.tensor.matmul(pl1[:, :], cm1w[:, k, :], px[:, k:k + 1],
                             start=(k == 0), stop=(k == KC - 1))
        cm1 = sm2.tile([128, 1], f32, name=f"cm1{s}", tag="cm1")
        nc.vector.tensor_scalar(out=cm1, in0=pl1[:, :], scalar1=cm1nb[:, :],
                                scalar2=None, op0=OP.is_gt)
        b1c = sm2.tile([128, 1], f32, name=f"b1c{s}", tag="b1c")
        nc.vector.tensor_mul(b1c, cm1, b1v)

        if KSTAGE < 3:
            nc.sync.dma_start(out=out_d[s, 0:128, :, :].rearrange("c h w -> c (h w)"),
                              in_=signmask[:, :])
            continue
        # ---------------- stage D: dilated mask ----------------
        t2d = dil1.tile([PW, PW], f32, name=f"t2d{s}", tag="t2d")
        nc.gpsimd.memset(t2d, 0.0)
        nc.sync.dma_start(out=t2d[1:57, 1:57], in_=signmask[0:1, :])
        hm1 = dil1.tile([PW, PW], f32, name=f"hm1{s}", tag="hm1")
        hm2 = dil1.tile([PW, PW], f32, name=f"hm2{s}", tag="hm2")
        nc.gpsimd.memset(hm1, 0.0)
        nc.gpsimd.memset(hm2, 0.0)
        nc.vector.tensor_tensor(out=hm1[:, 1:57], in0=t2d[:, 0:56], in1=t2d[:, 2:58],
                                op=OP.max)
        nc.vector.tensor_tensor(out=hm2[:, 1:57], in0=hm1[:, 1:57], in1=t2d[:, 1:57],
                                op=OP.max)
        vup = dil1.tile([PW, PW], f32, name=f"vup{s}", tag="vup")
        vdn = dil1.tile([PW, PW], f32, name=f"vdn{s}", tag="vdn")
        nc.gpsimd.memset(vup, 0.0)
        nc.gpsimd.memset(vdn, 0.0)
        nc.sync.dma_start(out=vup[0:57, 1:57], in_=hm2[1:58, 1:57])
        nc.sync.dma_start(out=vdn[1:58, 1:57], in_=hm2[0:57, 1:57])
        dl1 = dil1.tile([PW, PW], f32, name=f"dl1{s}", tag="dl1")
        dl2 = dil1.tile([PW, PW], f32, name=f"dl2{s}", tag="dl2")
        nc.vector.tensor_tensor(out=dl1, in0=hm2, in1=vup, op=OP.max)
        nc.vector.tensor_tensor(out=dl2, in0=dl1, in1=vdn, op=OP.max)
        # t2d is the 0/1 undilated mask in 2D; row/col sums feed the CA pools
        syc = sm2.tile([PW, 1], f32, name=f"syc{s}", tag="syc")
        nc.vector.tensor_reduce(out=syc, in_=t2d, axis=AX.X, op=OP.add)
        psx = pvec.tile([PW, 1], f32, name=f"psx{s}", tag="pvec")
        nc.tensor.matmul(psx[:, :], t2d[:, :], ones58[:, :], start=True, stop=True)
        sxc = sm2.tile([PW, 1], f32, name=f"sxc{s}", tag="sxc")
        nc.vector.tensor_copy(out=sxc, in_=psx[:, :])
        sy_row = sm2.tile([1, W], f32, name=f"sy_row{s}", tag="sy_row")
        sx_row = sm2.tile([1, W], f32, name=f"sx_row{s}", tag="sx_row")
        nc.sync.dma_start(out=sy_row, in_=syc[1:57, 0:1])
        nc.sync.dma_start(out=sx_row, in_=sxc[1:57, 0:1])
        dil_row = rows1.tile([1, NPIX], f32, name=f"dil_row{s}", tag="dil_row")
        nc.sync.dma_start(out=dil_row[0:1, :], in_=dl2[1:57, 1:57])
        sdil = big1.tile([128, NPIX], f32, name=f"sdil{s}", tag="sdil")
        nc.gpsimd.partition_broadcast(sdil[:, :], dil_row[:, :])

        if KSTAGE < 4:
            nc.sync.dma_start(out=out_d[s, 0:128, :, :].rearrange("c h w -> c (h w)"),
                              in_=sdil[:, :])
            continue
        # ---------------- stage E: conv1 ----------------
        h1m = big1.tile([128, PADN], f32, name=f"h1m{s}", tag="h1m")
        nc.gpsimd.memset(h1m, 0.0)
        h1m3 = h1m.rearrange("p (r c) -> p r c", r=PW)
        ph1 = sm2.tile([128, NT + 1], f32, name=f"ph1_{s}", tag="ph1_")
        for t in range(NT):
            ps = pbig.tile([128, TW], f32, name=f"ps_c1_{s}_{t}", tag="pbig")
            for k in range(KC):
                nc.tensor.matmul(ps[:, :], w1T[:, k, :], xk[k][:, TW * t:TW * (t + 1)],
                                 start=(k == 0), stop=(k == KC - 1))
            h1r = mid.tile([128, TW], f32, name=f"h1r{s}_{t}", tag="hr")
            nc.scalar.activation(out=h1r, in_=ps[:, :], func=AF.Relu,
                                 bias=b1c[:, :], scale=cm1[:, :])
            nc.vector.scalar_tensor_tensor(
                out=h1m3[:, 1 + RT * t:1 + RT * (t + 1), 1:57],
                in0=sdil[:, TW * t:TW * (t + 1)].rearrange("p (a b) -> p a b", a=RT),
                scalar=1.0,
                in1=h1r.rearrange("p (a b) -> p a b", a=RT),
                op0=OP.mult, op1=OP.mult,
                accum_out=ph1[:, t:t + 1])
        p1s = sm2.tile([128, 1], f32, name=f"p1s{s}", tag="p1s")
        nc.vector.tensor_reduce(out=p1s, in_=ph1[:, 0:NT], axis=AX.X, op=OP.add)
        pl2 = pvec.tile([128, 1], f32, name=f"pl2{s}", tag="pvec")
        nc.tensor.matmul(pl2[:, :], cm2w[:, :], p1s[:, :], start=True, stop=True)
        cm2 = sm2.tile([128, 1], f32, name=f"cm2{s}", tag="cm2")
        nc.vector.tensor_scalar(out=cm2, in0=pl2[:, :], scalar1=cm2nb[:, :],
                                scalar2=None, op0=OP.is_gt)
        b2c = sm2.tile([128, 1], f32, name=f"b2c{s}", tag="b2c")
        nc.vector.tensor_mul(b2c, cm2, b2v)

        if KSTAGE < 5:
            nc.sync.dma_start(out=out_d[s, 0:128, :, :].rearrange("c h w -> c (h w)"),
                              in_=h1m[:, 0:NPIX])
            continue
        # ---------------- stage G: conv2 ----------------
        ph2 = sm2.tile([128, NT + 1], f32, name=f"ph2_{s}", tag="ph2_")
        for t in range(NT):
            ps = pbig.tile([128, TW], f32, name=f"ps_c2_{s}_{t}", tag="pbig")
            first = True
            for dy in range(3):
                for dx in range(3):
                    nc.tensor.matmul(
                        ps[:, :], w2T[:, 3 * dy + dx, :],
                        h1m3[:, RT * t + dy:RT * t + dy + RT, dx:dx + 56],
                        start=first, stop=(dy == 2 and dx == 2))
                    first = False
            h2r = mid.tile([128, TW], f32, name=f"h2r{s}_{t}", tag="hr")
            nc.scalar.activation(out=h2r, in_=ps[:, :], func=AF.Relu,
                                 bias=b2c[:, :], scale=cm2[:, :])
            nc.vector.scalar_tensor_tensor(
                out=h2m[:, TW * t:TW * (t + 1)],
                in0=signmask[:, TW * t:TW * (t + 1)],
                scalar=1.0,
                in1=h2r[:, :],
                op0=OP.mult, op1=OP.mult,
                accum_out=ph2[:, t:t + 1])
        p2s = sm2.tile([128, 1], f32, name=f"p2s{s}", tag="p2s")
        nc.vector.tensor_reduce(out=p2s, in_=ph2[:, 0:NT], axis=AX.X, op=OP.add)

        if KSTAGE < 6:
            nc.sync.dma_start(out=out_d[s, 0:128, :, :].rearrange("c h w -> c (h w)"),
                              in_=h2m[:, :])
            continue
        # ---------------- stage H: cm3 + coord-attention vectors ----------
        cm3 = sm2.tile([128, MC], f32, name=f"cm3_{s}", tag="cm3_")
        for mc in range(MC):
            pl3 = pvec.tile([128, 1], f32, name=f"pl3{s}_{mc}", tag="pvec")
            nc.tensor.matmul(pl3[:, :], cm3w[:, mc, :], p2s[:, :], start=True, stop=True)
            nc.vector.tensor_scalar(out=cm3[:, mc:mc + 1], in0=pl3[:, :],
                                    scalar1=cm3nb[:, mc:mc + 1], scalar2=None,
                                    op0=OP.is_gt)
        xh_pre = sm2.tile([128, W], f32, name=f"xh_pre{s}", tag="xh_pre")
        xw_pre = sm2.tile([128, W], f32, name=f"xw_pre{s}", tag="xw_pre")
        nc.vector.tensor_reduce(out=xh_pre, in_=h2m.rearrange("p (y x) -> p y x", y=H),
                                axis=AX.X, op=OP.add)
        nc.vector.tensor_reduce(out=xw_pre, in_=h2m.rearrange("p (y x) -> p x y", y=H),
                                axis=AX.X, op=OP.add)
        xcat = sm2.tile([128, KC, 2 * W], f32, name=f"xcat{s}", tag="xcat")
        for mc in range(MC):
            pxh = pvec.tile([128, W], f32, name=f"pxh{s}_{mc}", tag="pvec")
            nc.tensor.matmul(pxh[:, :], w3T[:, mc, :], xh_pre[:, :], start=True, stop=False)
            nc.tensor.matmul(pxh[:, :], b3r[0:1, 128 * mc:128 * (mc + 1)], sy_row[:, :],
                             start=False, stop=True)
            nc.scalar.activation(out=xcat[:, mc, 0:W], in_=pxh[:, :], func=AF.Copy,
                                 scale=cm3[:, mc:mc + 1])
            pxw = pvec.tile([128, W], f32, name=f"pxw{s}_{mc}", tag="pvec")
            nc.tensor.matmul(pxw[:, :], w3T[:, mc, :], xw_pre[:, :], start=True, stop=False)
            nc.tensor.matmul(pxw[:, :], b3r[0:1, 128 * mc:128 * (mc + 1)], sx_row[:, :],
                             start=False, stop=True)
            nc.scalar.activation(out=xcat[:, mc, W:2 * W], in_=pxw[:, :], func=AF.Copy,
                                 scale=cm3[:, mc:mc + 1])
        py1 = pvec.tile([MIP, 2 * W], f32, name=f"py1{s}", tag="pvec")
        for k in range(KC):
            nc.tensor.matmul(py1[:, :], caw1[:, k, :], xcat[:, k, :],
                             start=(k == 0), stop=(k == KC - 1))
        r6 = sm2.tile([MIP, 2 * W], f32, name=f"r6_{s}", tag="r6_")
        nc.scalar.activation(out=r6, in_=py1[:, :], func=AF.Relu, bias=cab1p3[:, :])
        r6b = sm2.tile([MIP, 2 * W], f32, name=f"r6b{s}", tag="r6b")
        nc.vector.tensor_scalar(out=r6b, in0=r6, scalar1=6.0, scalar2=1.0 / 6.0,
                                op0=OP.min, op1=OP.mult)
        y1 = sm2.tile([MIP, 2 * W], f32, name=f"y1_{s}", tag="y1_")
        nc.vector.tensor_tensor(out=y1, in0=r6b, in1=py1[:, :], op=OP.mult)
        ahc = sm2.tile([128, MC, W], f32, name=f"ahc{s}", tag="ahc")
        awt = sm2.tile([128, MC, W], f32, name=f"awt{s}", tag="awt")
        for mc in range(MC):
            pah = pvec.tile([128, W], f32, name=f"pah{s}_{mc}", tag="pvec")
            nc.tensor.matmul(pah[:, :], cawh[:, mc, :], y1[:, 0:W], start=True, stop=True)
            aht = sm2.tile([128, W], f32, name=f"aht{s}_{mc}", tag="aht")
            nc.scalar.activation(out=aht, in_=pah[:, :], func=AF.Sigmoid,
                                 bias=cabh[:, mc:mc + 1])
            nc.vector.tensor_scalar(out=ahc[:, mc, :], in0=aht, scalar1=cm3[:, mc:mc + 1],
                                    scalar2=None, op0=OP.mult)
            paw = pvec.tile([128, W], f32, name=f"paw{s}_{mc}", tag="pvec")
            nc.tensor.matmul(paw[:, :], caww[:, mc, :], y1[:, W:2 * W], start=True, stop=True)
            nc.scalar.activation(out=awt[:, mc, :], in_=paw[:, :], func=AF.Sigmoid,
                                 bias=cabw[:, mc:mc + 1])

        if KSTAGE < 7:
            nc.sync.dma_start(out=out_d[s, 0:128, :, :].rearrange("c h w -> c (h w)"),
                              in_=h2m[:, :])
            continue
        # ------- stage I: conv3 + CA apply + residual + relu + store -------
        for mc in range(MC):
            for pair in range(NIT // 2):
                osb = outp.tile([128, 2 * IW], f32, name=f"osb{s}_{mc}_{pair}", tag="osb")
                for hp in range(2):
                    ht = 2 * pair + hp
                    pt = pc3.tile([128, 1024], f32, name=f"pt{s}_{mc}_{ht}", tag="pc3")
                    for hh in range(2):
                        lo = IW * ht + IH * hh
                        nc.tensor.matmul(pt[:, 512 * hh:512 * hh + IH], w3T[:, mc, :],
                                         h2m[:, lo:lo + IH], start=True, stop=False)
                        nc.tensor.matmul(pt[:, 512 * hh:512 * hh + IH],
                                         b3r[0:1, 128 * mc:128 * (mc + 1)],
                                         signmask[0:1, lo:lo + IH], start=False, stop=True)
                    ptv = pt[:, 0:1024].rearrange("p (h z) -> p h z", h=2)[:, :, 0:IH] \
                        .rearrange("p h (y x) -> p h y x", y=IR)
                    ahs = ahc[:, mc, 2 * IR * ht:2 * IR * (ht + 1)] \
                        .rearrange("p (h y) -> p h y", h=2).unsqueeze(3) \
                        .broadcast_to([128, 2, IR, W])
                    aws = awt[:, mc, :].unsqueeze(1).unsqueeze(1) \
                        .broadcast_to([128, 2, IR, W])
                    ut = mid.tile([128, IW], f32, name=f"ut{s}_{mc}_{ht}", tag="ut")
                    utv = ut.rearrange("p (h y x) -> p h y x", h=2, y=IR)
                    nc.vector.tensor_tensor(out=utv, in0=ptv, in1=ahs, op=OP.mult)
                    nc.vector.tensor_tensor(out=ptv, in0=utv, in1=aws, op=OP.mult)
                    for hh in range(2):
                        lo = IW * ht + IH * hh
                        nc.tensor.matmul(pt[:, 512 * hh:512 * hh + IH], idn[:, :],
                                         xk[mc][:, lo:lo + IH], start=False, stop=True,
                                         skip_group_check=True)
                    nc.scalar.activation(
                        out=osb[:, IW * hp:IW * hp + IW].rearrange(
                            "p (h y x) -> p h y x", h=2, y=IR),
                        in_=ptv, func=AF.Relu)
                nc.sync.dma_start(
                    out=out_d[s, 128 * mc:128 * (mc + 1), :, :].rearrange(
                        "c h w -> c (h w)")[:, 2 * IW * pair:2 * IW * (pair + 1)],
                    in_=osb[:, :])


def _build():
    nc = bacc.Bacc("TRN2", target_bir_lowering=False, debug=False)
    d = {}
    shapes = {
        "x": [BS, 512, H, W],
        "w1T": [128, KC, 128], "w2T": [128, 9, 128], "w3T": [128, MC, 128],
        "smw": [128, KC, 128], "cm1w": [128, KC, 128], "cm2w": [128, 128],
        "cm3w": [128, MC, 128], "caw1": [128, KC, MIP],
        "cawh": [MIP, MC, 128], "caww": [MIP, MC, 128],
        "idn": [128, 128], "b1v": [128, 1], "b2v": [128, 1], "b3r": [1, 512],
        "smbneg": [128, 1], "cm1nb": [128, 1], "cm2nb": [128, 1], "cm3nb": [128, MC],
        "cab1p3": [MIP, 1], "cabh": [128, MC], "cabw": [128, MC],
    }
    for name, shp in shapes.items():
        d[name] = nc.dram_tensor(name, shp, f32, kind="ExternalInput")
    d["out"] = nc.dram_tensor("out", [BS, 512, H, W], f32, kind="ExternalOutput")
    with tile.TileContext(nc) as tc, ExitStack() as ctx:
        _emit(nc, tc, ctx, d)
    nc.compile()
    return nc


def _prep_weights(i):
    """Host-side rearrangement of the full (replicated) weights."""
    N = NPIX
    w1 = i["w1"][:, :, 0, 0]                     # [128, 512]
    w3 = i["w3"][:, :, 0, 0]                     # [512, 128]
    caw1 = i["ca_w1"][:, :, 0, 0]                # [16, 512]
    cawh = i["ca_wh"][:, :, 0, 0]                # [512, 16]
    caww = i["ca_ww"][:, :, 0, 0]                # [512, 16]
    smv = i["sm_w"][0, :, 0, 0]                  # [512]
    f = np.float32
    return {
        "w1T": np.ascontiguousarray(w1.T.reshape(KC, 128, 128).transpose(1, 0, 2), f),
        "w2T": np.ascontiguousarray(i["w2"].transpose(1, 2, 3, 0).reshape(128, 9, 128), f),
        "w3T": np.ascontiguousarray(w3.T.reshape(128, MC, 128), f),
        "smw": np.ascontiguousarray(
            np.broadcast_to(smv.reshape(KC, 128).T[:, :, None], (128, KC, 128)), f),
        "cm1w": np.ascontiguousarray(
            (i["cm1_w"] / N).T.reshape(KC, 128, 128).transpose(1, 0, 2), f),
        "cm2w": np.ascontiguousarray((i["cm2_w"] / N).T, f),
        "cm3w": np.ascontiguousarray((i["cm3_w"] / N).T.reshape(128, MC, 128), f),
        "caw1": np.ascontiguousarray(
            (caw1 / W).T.reshape(KC, 128, MIP).transpose(1, 0, 2), f),
        "cawh": np.ascontiguousarray(cawh.T.reshape(MIP, MC, 128), f),
        "caww": np.ascontiguousarray(caww.T.reshape(MIP, MC, 128), f),
        "idn": np.eye(128, dtype=f),
        "b1v": i["b1"].reshape(128, 1).astype(f),
        "b2v": i["b2"].reshape(128, 1).astype(f),
        "b3r": i["b3"].reshape(1, 512).astype(f),
        "smbneg": np.full((128, 1), -i["sm_b"][0], f),
        "cm1nb": (-i["cm1_b"]).reshape(128, 1).astype(f),
        "cm2nb": (-i["cm2_b"]).reshape(128, 1).astype(f),
        "cm3nb": np.ascontiguousarray((-i["cm3_b"]).reshape(MC, 128).T, f),
        "cab1p3": (i["ca_b1"] + 3.0).reshape(MIP, 1).astype(f),
        "cabh": np.ascontiguousarray(i["ca_bh"].reshape(MC, 128).T, f),
        "cabw": np.ascontiguousarray(i["ca_bw"].reshape(MC, 128).T, f),
    }


_NC_CACHE = None


def _get_nc():
    global _NC_CACHE
    if _NC_CACHE is None:
        _NC_CACHE = _build()
    return _NC_CACHE


def kernel(**inputs):
    nc = _get_nc()
    wmap = _prep_weights(inputs)
    x = np.ascontiguousarray(inputs["x"], np.float32)
    in_maps = []
    for c in range(NCORES):
        m = dict(wmap)
        m["x"] = np.ascontiguousarray(x[BS * c:BS * (c + 1)])
        in_maps.append(m)
    res = run_bass_kernel_spmd(nc, in_maps, core_ids=list(range(NCORES)))
    return np.concatenate([r["out"] for r in res.results], axis=0)



# revision 77
# speedup vs baseline: 122.2759x; 122.2759x over previous
"""Trainium2 Bass kernel for the masked-bottleneck + coord-attention block.

Sharding: data-parallel over batch (B=16 -> 8 cores x 2 samples), weights
replicated. Everything below runs per-core on its 2-sample shard.

Per-sample dataflow (channels on partitions, 512 = 4 chunks of 128):
  s_logits   : PE matmul with sm_w replicated across all 128 output columns,
               so the [1,HW] logit row lands replicated on all partitions.
  signmask   : ACT Sign(logits + b) -> {-1,0,1} replicated mask (row 0 is the
               mask row). Dilated mask built on a [58,58] 2D view via tiny
               TT-max ops + partition-shift DMAs, then GPSIMD
               partition_broadcast back to [128,HW].
  conv1      : PE; epilogue ACT Relu(cm1*z + cm1*b1) then GPSIMD
               (sdil max 0) * h fused mask-multiply with free pooled sums.
  conv2      : 9 shifted-AP matmuls over the 58x58 zero-padded h1m.
  conv3+CA   : pools of h3 are derived by linearity (W3 @ pool(h2m) + b3*pool(s)),
               so h3 is never materialized: conv3 psum -> DVE *ah*cm3 -> DVE *aw
               -> PE accumulates identity@x (residual) -> ACT Relu -> out.
  channel masks: is_gt on tiny matmul outputs; pooled inputs come from ACT
               accum_out / STT accum_out side-channels.
"""

import os
import sys

for _p in ("/opt/trn_rl_repo", os.path.expanduser("~/.axon_site/_ro/trn_rl_repo")):
    if os.path.isdir(_p) and _p not in sys.path:
        sys.path.insert(0, _p)

import numpy as np
from contextlib import ExitStack, nullcontext

import concourse.bass as bass
from concourse import bacc
import concourse.mybir as mybir
import concourse.tile as tile
from concourse import library_config
from concourse.bass_utils import run_bass_kernel_spmd

f32 = mybir.dt.float32
AF = mybir.ActivationFunctionType
OP = mybir.AluOpType
AX = mybir.AxisListType

NCORES = 8
BS = 2                  # samples per core
KC = 4                  # 512 input channels -> 4 chunks of 128
MC = 4                  # 512 output channels -> 4 chunks of 128
WID = 128
MIP = 16
H = W = 56
NPIX = H * W            # 3136
PW = 58                 # zero-padded 2D side
PADN = PW * PW          # 3364
TW = 448                # slog/conv1/conv2 N-tile (8 rows)
NT = NPIX // TW         # 7
RT = TW // W            # 8 rows per tile
# stage I (conv3/CA/residual) tiling: 784 cols = 2 halves of 392 (7 rows each)
IW = 784
NIT = NPIX // IW        # 4
IH = 392                # half width (7 rows)
IR = 7                  # rows per half


KSTAGE = int(os.environ.get("KSTAGE", "9"))
KREPEAT = int(os.environ.get("KREPEAT", "1"))
# b3 is all-zero in setup_inputs(): the b3*smask matmuls add exact zeros, so
# skipping them leaves the output bit-identical while removing ~50k fp32 PE
# cycles per sample. kernel() sets this from the actual inputs.
B3ZERO = os.environ.get("B3ZERO", "1") == "1"


def _emit(nc, tc, ctx, d):
    sing = ctx.enter_context(tc.tile_pool(name="sing", bufs=1))
    xcp = ctx.enter_context(tc.tile_pool(name="xcp", bufs=4))
    big1 = ctx.enter_context(tc.tile_pool(name="big1", bufs=1))
    mid = ctx.enter_context(tc.tile_pool(name="mid", bufs=2))
    outp = ctx.enter_context(tc.tile_pool(name="outp", bufs=2))
    sm2 = ctx.enter_context(tc.tile_pool(name="sm2", bufs=2))
    dil1 = ctx.enter_context(tc.tile_pool(name="dil1", bufs=1))
    rows1 = ctx.enter_context(tc.tile_pool(name="rows1", bufs=1))
    pbig = ctx.enter_context(tc.tile_pool(name="pbig", bufs=2, space="PSUM"))
    pc3 = ctx.enter_context(tc.tile_pool(name="pc3", bufs=2, space="PSUM"))
    pvec = ctx.enter_context(tc.tile_pool(name="pvec", bufs=2, space="PSUM"))

    nc.gpsimd.load_library(library_config.mlp)

    # ---- weights / constants (loaded once) ----
    def wload(name, shape):
        t = sing.tile(shape, f32, name=name)
        nc.sync.dma_start(out=t, in_=d[name].ap())
        return t

    w1T = wload("w1T", [128, KC, 128])
    w2T = wload("w2T", [128, 9, 128])
    w3T = wload("w3T", [128, MC, 128])
    smw = wload("smw", [128, KC, 128])
    cm1w = wload("cm1w", [128, KC, 128])
    cm2w = wload("cm2w", [128, 128])
    cm3w = wload("cm3w", [128, MC, 128])
    caw1 = wload("caw1", [128, KC, MIP])
    cawh = wload("cawh", [MIP, MC, 128])
    caww = wload("caww", [MIP, MC, 128])
    idn = wload("idn", [128, 128])
    b1v = wload("b1v", [128, 1])
    b2v = wload("b2v", [128, 1])
    if not B3ZERO:
        b3r = wload("b3r", [1, 512])
    smbneg = wload("smbneg", [128, 1])
    cm1nb = wload("cm1nb", [128, 1])
    cm2nb = wload("cm2nb", [128, 1])
    cm3nb = wload("cm3nb", [128, MC])
    cab1p3 = wload("cab1p3", [MIP, 1])
    cabh = wload("cabh", [128, MC])
    cabw = wload("cabw", [128, MC])
    ones58 = sing.tile([PW, 1], f32)
    nc.vector.memset(ones58, 1.0)

    x_d = d["x"]
    out_d = d["out"]

    with (tc.For_i(0, KREPEAT, 1) if KREPEAT > 1 else nullcontext()):
      for s in range(BS):
        # ---------------- stage A: load x ----------------
        xk = []
        for k in range(KC):
            xt = xcp.tile([128, NPIX], f32, name=f"x_s{s}k{k}", tag="xc")
            nc.sync.dma_start(
                out=xt, in_=x_d[s, 128 * k:128 * (k + 1), :, :].rearrange("c h w -> c (h w)"))
            xk.append(xt)

        # ---------------- stage B: spatial-mask logits (replicated) ----------
        signmask = big1.tile([128, NPIX], f32, name=f"signmask{s}", tag="signmask")
        for t in range(NT):
            ps = pbig.tile([128, TW], f32, name=f"ps_slog{s}_{t}", tag="pbig")
            for k in range(KC):
                nc.tensor.matmul(ps[:, :], smw[:, k, :], xk[k][:, TW * t:TW * (t + 1)],
                                 start=(k == 0), stop=(k == KC - 1))
            nc.vector.tensor_scalar(out=signmask[:, TW * t:TW * (t + 1)], in0=ps[:, :],
                                    scalar1=smbneg[:, :], scalar2=None, op0=OP.is_gt)

        if KSTAGE < 2:
            nc.sync.dma_start(out=out_d[s, 0:128, :, :].rearrange("c h w -> c (h w)"),
                              in_=signmask[:, :])
            continue
        # ---------------- stage C: cm1 from pooled x ----------------
        px = sm2.tile([128, KC], f32, name=f"px{s}", tag="px")
        h2m = big1.tile([128, NPIX], f32, name=f"h2m{s}", tag="h2m")  # also px scratch
        for k in range(KC):
            nc.scalar.activation(out=h2m[:, :], in_=xk[k][:, :], func=AF.Copy,
                                 accum_out=px[:, k:k + 1])
        pl1 = pvec.tile([128, 1], f32, name=f"pl1{s}", tag="pvec")
        for k in range(KC):
            nc.tensor.matmul(pl1[:, :], cm1w[:, k, :], px[:, k:k + 1],
                             start=(k == 0), stop=(k == KC - 1))
        cm1 = sm2.tile([128, 1], f32, name=f"cm1{s}", tag="cm1")
        nc.vector.tensor_scalar(out=cm1, in0=pl1[:, :], scalar1=cm1nb[:, :],
                                scalar2=None, op0=OP.is_gt)
        b1c = sm2.tile([128, 1], f32, name=f"b1c{s}", tag="b1c")
        nc.vector.tensor_mul(b1c, cm1, b1v)

        if KSTAGE < 3:
            nc.sync.dma_start(out=out_d[s, 0:128, :, :].rearrange("c h w -> c (h w)"),
                              in_=signmask[:, :])
            continue
        # ---------------- stage D: dilated mask ----------------
        t2d = dil1.tile([PW, PW], f32, name=f"t2d{s}", tag="t2d")
        nc.gpsimd.memset(t2d, 0.0)
        nc.sync.dma_start(out=t2d[1:57, 1:57], in_=signmask[0:1, :])
        hm1 = dil1.tile([PW, PW], f32, name=f"hm1{s}", tag="hm1")
        hm2 = dil1.tile([PW, PW], f32, name=f"hm2{s}", tag="hm2")
        nc.gpsimd.memset(hm1, 0.0)
        nc.gpsimd.memset(hm2, 0.0)
        nc.vector.tensor_tensor(out=hm1[:, 1:57], in0=t2d[:, 0:56], in1=t2d[:, 2:58],
                                op=OP.max)
        nc.vector.tensor_tensor(out=hm2[:, 1:57], in0=hm1[:, 1:57], in1=t2d[:, 1:57],
                                op=OP.max)
        vup = dil1.tile([PW, PW], f32, name=f"vup{s}", tag="vup")
        vdn = dil1.tile([PW, PW], f32, name=f"vdn{s}", tag="vdn")
        nc.gpsimd.memset(vup, 0.0)
        nc.gpsimd.memset(vdn, 0.0)
        nc.sync.dma_start(out=vup[0:57, 1:57], in_=hm2[1:58, 1:57])
        nc.sync.dma_start(out=vdn[1:58, 1:57], in_=hm2[0:57, 1:57])
        dl1 = dil1.tile([PW, PW], f32, name=f"dl1{s}", tag="dl1")
        dl2 = dil1.tile([PW, PW], f32, name=f"dl2{s}", tag="dl2")
        nc.vector.tensor_tensor(out=dl1, in0=hm2, in1=vup, op=OP.max)
        nc.vector.tensor_tensor(out=dl2, in0=dl1, in1=vdn, op=OP.max)
        if not B3ZERO:
            # t2d row/col sums feed the b3*pool(smask) CA terms
            syc = sm2.tile([PW, 1], f32, name=f"syc{s}", tag="syc")
            nc.vector.tensor_reduce(out=syc, in_=t2d, axis=AX.X, op=OP.add)
            psx = pvec.tile([PW, 1], f32, name=f"psx{s}", tag="pvec")
            nc.tensor.matmul(psx[:, :], t2d[:, :], ones58[:, :], start=True, stop=True)
            sxc = sm2.tile([PW, 1], f32, name=f"sxc{s}", tag="sxc")
            nc.vector.tensor_copy(out=sxc, in_=psx[:, :])
            sy_row = sm2.tile([1, W], f32, name=f"sy_row{s}", tag="sy_row")
            sx_row = sm2.tile([1, W], f32, name=f"sx_row{s}", tag="sx_row")
            nc.sync.dma_start(out=sy_row, in_=syc[1:57, 0:1])
            nc.sync.dma_start(out=sx_row, in_=sxc[1:57, 0:1])
        dil_row = rows1.tile([1, NPIX], f32, name=f"dil_row{s}", tag="dil_row")
        nc.sync.dma_start(out=dil_row[0:1, :], in_=dl2[1:57, 1:57])
        sdil = big1.tile([128, NPIX], f32, name=f"sdil{s}", tag="sdil")
        nc.gpsimd.partition_broadcast(sdil[:, :], dil_row[:, :])

        if KSTAGE < 4:
            nc.sync.dma_start(out=out_d[s, 0:128, :, :].rearrange("c h w -> c (h w)"),
                              in_=sdil[:, :])
            continue
        # ---------------- stage E: conv1 ----------------
        h1m = big1.tile([128, PADN], f32, name=f"h1m{s}", tag="h1m")
        nc.gpsimd.memset(h1m, 0.0)
        h1m3 = h1m.rearrange("p (r c) -> p r c", r=PW)
        ph1 = sm2.tile([128, NT + 1], f32, name=f"ph1_{s}", tag="ph1_")
        for t in range(NT):
            ps = pbig.tile([128, TW], f32, name=f"ps_c1_{s}_{t}", tag="pbig")
            for k in range(KC):
                nc.tensor.matmul(ps[:, :], w1T[:, k, :], xk[k][:, TW * t:TW * (t + 1)],
                                 start=(k == 0), stop=(k == KC - 1))
            h1r = mid.tile([128, TW], f32, name=f"h1r{s}_{t}", tag="hr")
            nc.scalar.activation(out=h1r, in_=ps[:, :], func=AF.Relu,
                                 bias=b1c[:, :], scale=cm1[:, :])
            nc.vector.scalar_tensor_tensor(
                out=h1m3[:, 1 + RT * t:1 + RT * (t + 1), 1:57],
                in0=sdil[:, TW * t:TW * (t + 1)].rearrange("p (a b) -> p a b", a=RT),
                scalar=1.0,
                in1=h1r.rearrange("p (a b) -> p a b", a=RT),
                op0=OP.mult, op1=OP.mult,
                accum_out=ph1[:, t:t + 1])
        p1s = sm2.tile([128, 1], f32, name=f"p1s{s}", tag="p1s")
        nc.vector.tensor_reduce(out=p1s, in_=ph1[:, 0:NT], axis=AX.X, op=OP.add)
        pl2 = pvec.tile([128, 1], f32, name=f"pl2{s}", tag="pvec")
        nc.tensor.matmul(pl2[:, :], cm2w[:, :], p1s[:, :], start=True, stop=True)
        cm2 = sm2.tile([128, 1], f32, name=f"cm2{s}", tag="cm2")
        nc.vector.tensor_scalar(out=cm2, in0=pl2[:, :], scalar1=cm2nb[:, :],
                                scalar2=None, op0=OP.is_gt)
        b2c = sm2.tile([128, 1], f32, name=f"b2c{s}", tag="b2c")
        nc.vector.tensor_mul(b2c, cm2, b2v)

        if KSTAGE < 5:
            nc.sync.dma_start(out=out_d[s, 0:128, :, :].rearrange("c h w -> c (h w)"),
                              in_=h1m[:, 0:NPIX])
            continue
        # ---------------- stage G: conv2 ----------------
        ph2 = sm2.tile([128, NT + 1], f32, name=f"ph2_{s}", tag="ph2_")
        for t in range(NT):
            ps = pbig.tile([128, TW], f32, name=f"ps_c2_{s}_{t}", tag="pbig")
            first = True
            for dy in range(3):
                for dx in range(3):
                    nc.tensor.matmul(
                        ps[:, :], w2T[:, 3 * dy + dx, :],
                        h1m3[:, RT * t + dy:RT * t + dy + RT, dx:dx + 56],
                        start=first, stop=(dy == 2 and dx == 2))
                    first = False
            h2r = mid.tile([128, TW], f32, name=f"h2r{s}_{t}", tag="hr")
            nc.scalar.activation(out=h2r, in_=ps[:, :], func=AF.Relu,
                                 bias=b2c[:, :], scale=cm2[:, :])
            nc.vector.scalar_tensor_tensor(
                out=h2m[:, TW * t:TW * (t + 1)],
                in0=signmask[:, TW * t:TW * (t + 1)],
                scalar=1.0,
                in1=h2r[:, :],
                op0=OP.mult, op1=OP.mult,
                accum_out=ph2[:, t:t + 1])
        p2s = sm2.tile([128, 1], f32, name=f"p2s{s}", tag="p2s")
        nc.vector.tensor_reduce(out=p2s, in_=ph2[:, 0:NT], axis=AX.X, op=OP.add)

        if KSTAGE < 6:
            nc.sync.dma_start(out=out_d[s, 0:128, :, :].rearrange("c h w -> c (h w)"),
                              in_=h2m[:, :])
            continue
        # ---------------- stage H: cm3 + coord-attention vectors ----------
        cm3 = sm2.tile([128, MC], f32, name=f"cm3_{s}", tag="cm3_")
        for mc in range(MC):
            pl3 = pvec.tile([128, 1], f32, name=f"pl3{s}_{mc}", tag="pvec")
            nc.tensor.matmul(pl3[:, :], cm3w[:, mc, :], p2s[:, :], start=True, stop=True)
            nc.vector.tensor_scalar(out=cm3[:, mc:mc + 1], in0=pl3[:, :],
                                    scalar1=cm3nb[:, mc:mc + 1], scalar2=None,
                                    op0=OP.is_gt)
        xh_pre = sm2.tile([128, W], f32, name=f"xh_pre{s}", tag="xh_pre")
        xw_pre = sm2.tile([128, W], f32, name=f"xw_pre{s}", tag="xw_pre")
        nc.vector.tensor_reduce(out=xh_pre, in_=h2m.rearrange("p (y x) -> p y x", y=H),
                                axis=AX.X, op=OP.add)
        nc.vector.tensor_reduce(out=xw_pre, in_=h2m.rearrange("p (y x) -> p x y", y=H),
                                axis=AX.X, op=OP.add)
        xcat = sm2.tile([128, KC, 2 * W], f32, name=f"xcat{s}", tag="xcat")
        for mc in range(MC):
            pxh = pvec.tile([128, W], f32, name=f"pxh{s}_{mc}", tag="pvec")
            nc.tensor.matmul(pxh[:, :], w3T[:, mc, :], xh_pre[:, :], start=True,
                             stop=B3ZERO)
            if not B3ZERO:
                nc.tensor.matmul(pxh[:, :], b3r[0:1, 128 * mc:128 * (mc + 1)],
                                 sy_row[:, :], start=False, stop=True)
            nc.scalar.activation(out=xcat[:, mc, 0:W], in_=pxh[:, :], func=AF.Copy,
                                 scale=cm3[:, mc:mc + 1])
            pxw = pvec.tile([128, W], f32, name=f"pxw{s}_{mc}", tag="pvec")
            nc.tensor.matmul(pxw[:, :], w3T[:, mc, :], xw_pre[:, :], start=True,
                             stop=B3ZERO)
            if not B3ZERO:
                nc.tensor.matmul(pxw[:, :], b3r[0:1, 128 * mc:128 * (mc + 1)],
                                 sx_row[:, :], start=False, stop=True)
            nc.scalar.activation(out=xcat[:, mc, W:2 * W], in_=pxw[:, :], func=AF.Copy,
                                 scale=cm3[:, mc:mc + 1])
        py1 = pvec.tile([MIP, 2 * W], f32, name=f"py1{s}", tag="pvec")
        for k in range(KC):
            nc.tensor.matmul(py1[:, :], caw1[:, k, :], xcat[:, k, :],
                             start=(k == 0), stop=(k == KC - 1))
        r6 = sm2.tile([MIP, 2 * W], f32, name=f"r6_{s}", tag="r6_")
        nc.scalar.activation(out=r6, in_=py1[:, :], func=AF.Relu, bias=cab1p3[:, :])
        r6b = sm2.tile([MIP, 2 * W], f32, name=f"r6b{s}", tag="r6b")
        nc.vector.tensor_scalar(out=r6b, in0=r6, scalar1=6.0, scalar2=1.0 / 6.0,
                                op0=OP.min, op1=OP.mult)
        y1 = sm2.tile([MIP, 2 * W], f32, name=f"y1_{s}", tag="y1_")
        nc.vector.tensor_tensor(out=y1, in0=r6b, in1=py1[:, :], op=OP.mult)
        ahc = sm2.tile([128, MC, W], f32, name=f"ahc{s}", tag="ahc")
        awt = sm2.tile([128, MC, W], f32, name=f"awt{s}", tag="awt")
        for mc in range(MC):
            pah = pvec.tile([128, W], f32, name=f"pah{s}_{mc}", tag="pvec")
            nc.tensor.matmul(pah[:, :], cawh[:, mc, :], y1[:, 0:W], start=True, stop=True)
            aht = sm2.tile([128, W], f32, name=f"aht{s}_{mc}", tag="aht")
            nc.scalar.activation(out=aht, in_=pah[:, :], func=AF.Sigmoid,
                                 bias=cabh[:, mc:mc + 1])
            nc.vector.tensor_scalar(out=ahc[:, mc, :], in0=aht, scalar1=cm3[:, mc:mc + 1],
                                    scalar2=None, op0=OP.mult)
            paw = pvec.tile([128, W], f32, name=f"paw{s}_{mc}", tag="pvec")
            nc.tensor.matmul(paw[:, :], caww[:, mc, :], y1[:, W:2 * W], start=True, stop=True)
            nc.scalar.activation(out=awt[:, mc, :], in_=paw[:, :], func=AF.Sigmoid,
                                 bias=cabw[:, mc:mc + 1])

        if KSTAGE < 7:
            nc.sync.dma_start(out=out_d[s, 0:128, :, :].rearrange("c h w -> c (h w)"),
                              in_=h2m[:, :])
            continue
        # ------- stage I: conv3 + CA apply + residual + relu + store -------
        for mc in range(MC):
            for pair in range(NIT // 2):
                osb = outp.tile([128, 2 * IW], f32, name=f"osb{s}_{mc}_{pair}", tag="osb")
                for hp in range(2):
                    ht = 2 * pair + hp
                    pt = pc3.tile([128, 1024], f32, name=f"pt{s}_{mc}_{ht}", tag="pc3")
                    for hh in range(2):
                        lo = IW * ht + IH * hh
                        nc.tensor.matmul(pt[:, 512 * hh:512 * hh + IH], w3T[:, mc, :],
                                         h2m[:, lo:lo + IH], start=True, stop=B3ZERO)
                        if not B3ZERO:
                            nc.tensor.matmul(pt[:, 512 * hh:512 * hh + IH],
                                             b3r[0:1, 128 * mc:128 * (mc + 1)],
                                             signmask[0:1, lo:lo + IH],
                                             start=False, stop=True)
                    ptv = pt[:, 0:1024].rearrange("p (h z) -> p h z", h=2)[:, :, 0:IH] \
                        .rearrange("p h (y x) -> p h y x", y=IR)
                    ahs = ahc[:, mc, 2 * IR * ht:2 * IR * (ht + 1)] \
                        .rearrange("p (h y) -> p h y", h=2).unsqueeze(3) \
                        .broadcast_to([128, 2, IR, W])
                    aws = awt[:, mc, :].unsqueeze(1).unsqueeze(1) \
                        .broadcast_to([128, 2, IR, W])
                    ut = mid.tile([128, IW], f32, name=f"ut{s}_{mc}_{ht}", tag="ut")
                    utv = ut.rearrange("p (h y x) -> p h y x", h=2, y=IR)
                    nc.vector.tensor_tensor(out=utv, in0=ptv, in1=ahs, op=OP.mult)
                    nc.vector.tensor_tensor(out=ptv, in0=utv, in1=aws, op=OP.mult)
                    for hh in range(2):
                        lo = IW * ht + IH * hh
                        nc.tensor.matmul(pt[:, 512 * hh:512 * hh + IH], idn[:, :],
                                         xk[mc][:, lo:lo + IH], start=False, stop=True,
                                         skip_group_check=True)
                    nc.scalar.activation(
                        out=osb[:, IW * hp:IW * hp + IW].rearrange(
                            "p (h y x) -> p h y x", h=2, y=IR),
                        in_=ptv, func=AF.Relu)
                nc.sync.dma_start(
                    out=out_d[s, 128 * mc:128 * (mc + 1), :, :].rearrange(
                        "c h w -> c (h w)")[:, 2 * IW * pair:2 * IW * (pair + 1)],
                    in_=osb[:, :])


def _build():
    nc = bacc.Bacc("TRN2", target_bir_lowering=False, debug=False)
    d = {}
    shapes = {
        "x": [BS, 512, H, W],
        "w1T": [128, KC, 128], "w2T": [128, 9, 128], "w3T": [128, MC, 128],
        "smw": [128, KC, 128], "cm1w": [128, KC, 128], "cm2w": [128, 128],
        "cm3w": [128, MC, 128], "caw1": [128, KC, MIP],
        "cawh": [MIP, MC, 128], "caww": [MIP, MC, 128],
        "idn": [128, 128], "b1v": [128, 1], "b2v": [128, 1], "b3r": [1, 512],
        "smbneg": [128, 1], "cm1nb": [128, 1], "cm2nb": [128, 1], "cm3nb": [128, MC],
        "cab1p3": [MIP, 1], "cabh": [128, MC], "cabw": [128, MC],
    }
    for name, shp in shapes.items():
        d[name] = nc.dram_tensor(name, shp, f32, kind="ExternalInput")
    d["out"] = nc.dram_tensor("out", [BS, 512, H, W], f32, kind="ExternalOutput")
    with tile.TileContext(nc) as tc, ExitStack() as ctx:
        _emit(nc, tc, ctx, d)
    nc.compile()
    return nc


def _prep_weights(i):
    """Host-side rearrangement of the full (replicated) weights."""
    N = NPIX
    w1 = i["w1"][:, :, 0, 0]                     # [128, 512]
    w3 = i["w3"][:, :, 0, 0]                     # [512, 128]
    caw1 = i["ca_w1"][:, :, 0, 0]                # [16, 512]
    cawh = i["ca_wh"][:, :, 0, 0]                # [512, 16]
    caww = i["ca_ww"][:, :, 0, 0]                # [512, 16]
    smv = i["sm_w"][0, :, 0, 0]                  # [512]
    f = np.float32
    return {
        "w1T": np.ascontiguousarray(w1.T.reshape(KC, 128, 128).transpose(1, 0, 2), f),
        "w2T": np.ascontiguousarray(i["w2"].transpose(1, 2, 3, 0).reshape(128, 9, 128), f),
        "w3T": np.ascontiguousarray(w3.T.reshape(128, MC, 128), f),
        "smw": np.ascontiguousarray(
            np.broadcast_to(smv.reshape(KC, 128).T[:, :, None], (128, KC, 128)), f),
        "cm1w": np.ascontiguousarray(
            (i["cm1_w"] / N).T.reshape(KC, 128, 128).transpose(1, 0, 2), f),
        "cm2w": np.ascontiguousarray((i["cm2_w"] / N).T, f),
        "cm3w": np.ascontiguousarray((i["cm3_w"] / N).T.reshape(128, MC, 128), f),
        "caw1": np.ascontiguousarray(
            (caw1 / W).T.reshape(KC, 128, MIP).transpose(1, 0, 2), f),
        "cawh": np.ascontiguousarray(cawh.T.reshape(MIP, MC, 128), f),
        "caww": np.ascontiguousarray(caww.T.reshape(MIP, MC, 128), f),
        "idn": np.eye(128, dtype=f),
        "b1v": i["b1"].reshape(128, 1).astype(f),
        "b2v": i["b2"].reshape(128, 1).astype(f),
        "b3r": i["b3"].reshape(1, 512).astype(f),
        "smbneg": np.full((128, 1), -i["sm_b"][0], f),
        "cm1nb": (-i["cm1_b"]).reshape(128, 1).astype(f),
        "cm2nb": (-i["cm2_b"]).reshape(128, 1).astype(f),
        "cm3nb": np.ascontiguousarray((-i["cm3_b"]).reshape(MC, 128).T, f),
        "cab1p3": (i["ca_b1"] + 3.0).reshape(MIP, 1).astype(f),
        "cabh": np.ascontiguousarray(i["ca_bh"].reshape(MC, 128).T, f),
        "cabw": np.ascontiguousarray(i["ca_bw"].reshape(MC, 128).T, f),
    }


_NC_CACHE = {}


def _get_nc():
    key = (B3ZERO, KREPEAT)
    if key not in _NC_CACHE:
        _NC_CACHE[key] = _build()
    return _NC_CACHE[key]


def kernel(**inputs):
    global B3ZERO
    B3ZERO = not np.any(inputs["b3"])
    nc = _get_nc()
    wmap = _prep_weights(inputs)
    x = np.ascontiguousarray(inputs["x"], np.float32)
    in_maps = []
    for c in range(NCORES):
        m = dict(wmap)
        m["x"] = np.ascontiguousarray(x[BS * c:BS * (c + 1)])
        in_maps.append(m)
    res = run_bass_kernel_spmd(nc, in_maps, core_ids=list(range(NCORES)))
    return np.concatenate([r["out"] for r in res.results], axis=0)


# revision 81
# speedup vs baseline: 189.3566x; 1.5486x over previous
"""Trainium2 Bass kernel for the masked-bottleneck + coord-attention block.

Sharding: data-parallel over batch (B=16 -> 8 cores x 2 samples), weights
replicated. Everything below runs per-core on its 2-sample shard.

Per-sample dataflow (channels on partitions, 512 = 4 chunks of 128):
  s_logits   : PE matmul with sm_w replicated across all 128 output columns,
               so the [1,HW] logit row lands replicated on all partitions.
  signmask   : ACT Sign(logits + b) -> {-1,0,1} replicated mask (row 0 is the
               mask row). Dilated mask built on a [58,58] 2D view via tiny
               TT-max ops + partition-shift DMAs, then GPSIMD
               partition_broadcast back to [128,HW].
  conv1      : PE; epilogue ACT Relu(cm1*z + cm1*b1) then GPSIMD
               (sdil max 0) * h fused mask-multiply with free pooled sums.
  conv2      : 9 shifted-AP matmuls over the 58x58 zero-padded h1m.
  conv3+CA   : pools of h3 are derived by linearity (W3 @ pool(h2m) + b3*pool(s)),
               so h3 is never materialized: conv3 psum -> DVE *ah*cm3 -> DVE *aw
               -> PE accumulates identity@x (residual) -> ACT Relu -> out.
  channel masks: is_gt on tiny matmul outputs; pooled inputs come from ACT
               accum_out / STT accum_out side-channels.
"""

import os
import sys

for _p in ("/opt/trn_rl_repo", os.path.expanduser("~/.axon_site/_ro/trn_rl_repo")):
    if os.path.isdir(_p) and _p not in sys.path:
        sys.path.insert(0, _p)

import numpy as np
from contextlib import ExitStack, nullcontext

import concourse.bass as bass
from concourse import bacc
import concourse.mybir as mybir
import concourse.tile as tile
from concourse import library_config
from concourse.bass_utils import run_bass_kernel_spmd

f32 = mybir.dt.float32
AF = mybir.ActivationFunctionType
OP = mybir.AluOpType
AX = mybir.AxisListType

NCORES = 8
BS = 2                  # samples per core
KC = 4                  # 512 input channels -> 4 chunks of 128
MC = 4                  # 512 output channels -> 4 chunks of 128
WID = 128
MIP = 16
H = W = 56
NPIX = H * W            # 3136
PW = 58                 # zero-padded 2D side
PADN = PW * PW          # 3364
TW = 448                # slog/conv1/conv2 N-tile (8 rows)
NT = NPIX // TW         # 7
RT = TW // W            # 8 rows per tile
# stage I (conv3/CA/residual) tiling: 784 cols = 2 halves of 392 (7 rows each)
IW = 784
NIT = NPIX // IW        # 4
IH = 392                # half width (7 rows)
IR = 7                  # rows per half


KSTAGE = int(os.environ.get("KSTAGE", "9"))
KREPEAT = int(os.environ.get("KREPEAT", "1"))
# b3 is all-zero in setup_inputs(): the b3*smask matmuls add exact zeros, so
# skipping them leaves the output bit-identical while removing ~50k fp32 PE
# cycles per sample. kernel() sets this from the actual inputs.
B3ZERO = os.environ.get("B3ZERO", "1") == "1"


def _emit(nc, tc, ctx, d):
    sing = ctx.enter_context(tc.tile_pool(name="sing", bufs=1))
    xcp = ctx.enter_context(tc.tile_pool(name="xcp", bufs=7))
    big1 = ctx.enter_context(tc.tile_pool(name="big1", bufs=1))
    mid = ctx.enter_context(tc.tile_pool(name="mid", bufs=2))
    outp = ctx.enter_context(tc.tile_pool(name="outp", bufs=2))
    sm2 = ctx.enter_context(tc.tile_pool(name="sm2", bufs=2))
    dil1 = ctx.enter_context(tc.tile_pool(name="dil1", bufs=1))
    rows1 = ctx.enter_context(tc.tile_pool(name="rows1", bufs=1))
    pbig = ctx.enter_context(tc.tile_pool(name="pbig", bufs=2, space="PSUM"))
    pc3 = ctx.enter_context(tc.tile_pool(name="pc3", bufs=2, space="PSUM"))
    pvec = ctx.enter_context(tc.tile_pool(name="pvec", bufs=2, space="PSUM"))

    nc.gpsimd.load_library(library_config.mlp)

    # ---- weights / constants (loaded once) ----
    def wload(name, shape):
        t = sing.tile(shape, f32, name=name)
        nc.sync.dma_start(out=t, in_=d[name].ap())
        return t

    w1T = wload("w1T", [128, KC, 128])
    w2T = wload("w2T", [128, 9, 128])
    w3T = wload("w3T", [128, MC, 128])
    smw = wload("smw", [128, KC, 128])
    cm1w = wload("cm1w", [128, KC, 128])
    cm2w = wload("cm2w", [128, 128])
    cm3w = wload("cm3w", [128, MC, 128])
    caw1 = wload("caw1", [128, KC, MIP])
    cawh = wload("cawh", [MIP, MC, 128])
    caww = wload("caww", [MIP, MC, 128])
    idn = wload("idn", [128, 128])
    b1v = wload("b1v", [128, 1])
    b2v = wload("b2v", [128, 1])
    if not B3ZERO:
        b3r = wload("b3r", [1, 512])
    smbneg = wload("smbneg", [128, 1])
    cm1nb = wload("cm1nb", [128, 1])
    cm2nb = wload("cm2nb", [128, 1])
    cm3nb = wload("cm3nb", [128, MC])
    cab1p3 = wload("cab1p3", [MIP, 1])
    cabh = wload("cabh", [128, MC])
    cabw = wload("cabw", [128, MC])
    ones58 = sing.tile([PW, 1], f32)
    nc.vector.memset(ones58, 1.0)

    x_d = d["x"]
    out_d = d["out"]

    with (tc.For_i(0, KREPEAT, 1) if KREPEAT > 1 else nullcontext()):
      for s in range(BS):
        # ---------------- stage A: load x ----------------
        xk = []
        for k in range(KC):
            xt = xcp.tile([128, NPIX], f32, name=f"x_s{s}k{k}", tag="xc")
            nc.sync.dma_start(
                out=xt, in_=x_d[s, 128 * k:128 * (k + 1), :, :].rearrange("c h w -> c (h w)"))
            xk.append(xt)

        # ---------------- stage B: spatial-mask logits (replicated) ----------
        signmask = big1.tile([128, NPIX], f32, name=f"signmask{s}", tag="signmask")
        for t in range(NT):
            ps = pbig.tile([128, TW], f32, name=f"ps_slog{s}_{t}", tag="pbig")
            for k in range(KC):
                nc.tensor.matmul(ps[:, :], smw[:, k, :], xk[k][:, TW * t:TW * (t + 1)],
                                 start=(k == 0), stop=(k == KC - 1))
            nc.vector.tensor_scalar(out=signmask[:, TW * t:TW * (t + 1)], in0=ps[:, :],
                                    scalar1=smbneg[:, :], scalar2=None, op0=OP.is_gt)

        if KSTAGE < 2:
            nc.sync.dma_start(out=out_d[s, 0:128, :, :].rearrange("c h w -> c (h w)"),
                              in_=signmask[:, :])
            continue
        # ---------------- stage C: cm1 from pooled x ----------------
        px = sm2.tile([128, KC], f32, name=f"px{s}", tag="px")
        h2m = big1.tile([128, NPIX], f32, name=f"h2m{s}", tag="h2m")  # also px scratch
        for k in range(KC):
            nc.scalar.activation(out=h2m[:, :], in_=xk[k][:, :], func=AF.Copy,
                                 accum_out=px[:, k:k + 1])
        pl1 = pvec.tile([128, 1], f32, name=f"pl1{s}", tag="pvec")
        for k in range(KC):
            nc.tensor.matmul(pl1[:, :], cm1w[:, k, :], px[:, k:k + 1],
                             start=(k == 0), stop=(k == KC - 1))
        cm1 = sm2.tile([128, 1], f32, name=f"cm1{s}", tag="cm1")
        nc.vector.tensor_scalar(out=cm1, in0=pl1[:, :], scalar1=cm1nb[:, :],
                                scalar2=None, op0=OP.is_gt)
        b1c = sm2.tile([128, 1], f32, name=f"b1c{s}", tag="b1c")
        nc.vector.tensor_mul(b1c, cm1, b1v)

        if KSTAGE < 3:
            nc.sync.dma_start(out=out_d[s, 0:128, :, :].rearrange("c h w -> c (h w)"),
                              in_=signmask[:, :])
            continue
        # ---------------- stage D: dilated mask ----------------
        t2d = dil1.tile([PW, PW], f32, name=f"t2d{s}", tag="t2d")
        nc.gpsimd.memset(t2d, 0.0)
        nc.sync.dma_start(out=t2d[1:57, 1:57], in_=signmask[0:1, :])
        hm1 = dil1.tile([PW, PW], f32, name=f"hm1{s}", tag="hm1")
        hm2 = dil1.tile([PW, PW], f32, name=f"hm2{s}", tag="hm2")
        nc.gpsimd.memset(hm1, 0.0)
        nc.gpsimd.memset(hm2, 0.0)
        nc.vector.tensor_tensor(out=hm1[:, 1:57], in0=t2d[:, 0:56], in1=t2d[:, 2:58],
                                op=OP.max)
        nc.vector.tensor_tensor(out=hm2[:, 1:57], in0=hm1[:, 1:57], in1=t2d[:, 1:57],
                                op=OP.max)
        vup = dil1.tile([PW, PW], f32, name=f"vup{s}", tag="vup")
        vdn = dil1.tile([PW, PW], f32, name=f"vdn{s}", tag="vdn")
        nc.gpsimd.memset(vup, 0.0)
        nc.gpsimd.memset(vdn, 0.0)
        nc.sync.dma_start(out=vup[0:57, 1:57], in_=hm2[1:58, 1:57])
        nc.sync.dma_start(out=vdn[1:58, 1:57], in_=hm2[0:57, 1:57])
        dl1 = dil1.tile([PW, PW], f32, name=f"dl1{s}", tag="dl1")
        dl2 = dil1.tile([PW, PW], f32, name=f"dl2{s}", tag="dl2")
        nc.vector.tensor_tensor(out=dl1, in0=hm2, in1=vup, op=OP.max)
        nc.vector.tensor_tensor(out=dl2, in0=dl1, in1=vdn, op=OP.max)
        if not B3ZERO:
            # t2d row/col sums feed the b3*pool(smask) CA terms
            syc = sm2.tile([PW, 1], f32, name=f"syc{s}", tag="syc")
            nc.vector.tensor_reduce(out=syc, in_=t2d, axis=AX.X, op=OP.add)
            psx = pvec.tile([PW, 1], f32, name=f"psx{s}", tag="pvec")
            nc.tensor.matmul(psx[:, :], t2d[:, :], ones58[:, :], start=True, stop=True)
            sxc = sm2.tile([PW, 1], f32, name=f"sxc{s}", tag="sxc")
            nc.vector.tensor_copy(out=sxc, in_=psx[:, :])
            sy_row = sm2.tile([1, W], f32, name=f"sy_row{s}", tag="sy_row")
            sx_row = sm2.tile([1, W], f32, name=f"sx_row{s}", tag="sx_row")
            nc.sync.dma_start(out=sy_row, in_=syc[1:57, 0:1])
            nc.sync.dma_start(out=sx_row, in_=sxc[1:57, 0:1])
        dil_row = rows1.tile([1, NPIX], f32, name=f"dil_row{s}", tag="dil_row")
        nc.sync.dma_start(out=dil_row[0:1, :], in_=dl2[1:57, 1:57])
        sdil = big1.tile([128, NPIX], f32, name=f"sdil{s}", tag="sdil")
        nc.gpsimd.partition_broadcast(sdil[:, :], dil_row[:, :])

        if KSTAGE < 4:
            nc.sync.dma_start(out=out_d[s, 0:128, :, :].rearrange("c h w -> c (h w)"),
                              in_=sdil[:, :])
            continue
        # ---------------- stage E: conv1 ----------------
        h1m = big1.tile([128, PADN], f32, name=f"h1m{s}", tag="h1m")
        nc.gpsimd.memset(h1m, 0.0)
        h1m3 = h1m.rearrange("p (r c) -> p r c", r=PW)
        ph1 = sm2.tile([128, NT + 1], f32, name=f"ph1_{s}", tag="ph1_")
        for t in range(NT):
            ps = pbig.tile([128, TW], f32, name=f"ps_c1_{s}_{t}", tag="pbig")
            for k in range(KC):
                nc.tensor.matmul(ps[:, :], w1T[:, k, :], xk[k][:, TW * t:TW * (t + 1)],
                                 start=(k == 0), stop=(k == KC - 1))
            h1r = mid.tile([128, TW], f32, name=f"h1r{s}_{t}", tag="hr")
            nc.scalar.activation(out=h1r, in_=ps[:, :], func=AF.Relu,
                                 bias=b1c[:, :], scale=cm1[:, :])
            nc.vector.scalar_tensor_tensor(
                out=h1m3[:, 1 + RT * t:1 + RT * (t + 1), 1:57],
                in0=sdil[:, TW * t:TW * (t + 1)].rearrange("p (a b) -> p a b", a=RT),
                scalar=1.0,
                in1=h1r.rearrange("p (a b) -> p a b", a=RT),
                op0=OP.mult, op1=OP.mult,
                accum_out=ph1[:, t:t + 1])
        p1s = sm2.tile([128, 1], f32, name=f"p1s{s}", tag="p1s")
        nc.vector.tensor_reduce(out=p1s, in_=ph1[:, 0:NT], axis=AX.X, op=OP.add)
        pl2 = pvec.tile([128, 1], f32, name=f"pl2{s}", tag="pvec")
        nc.tensor.matmul(pl2[:, :], cm2w[:, :], p1s[:, :], start=True, stop=True)
        cm2 = sm2.tile([128, 1], f32, name=f"cm2{s}", tag="cm2")
        nc.vector.tensor_scalar(out=cm2, in0=pl2[:, :], scalar1=cm2nb[:, :],
                                scalar2=None, op0=OP.is_gt)
        b2c = sm2.tile([128, 1], f32, name=f"b2c{s}", tag="b2c")
        nc.vector.tensor_mul(b2c, cm2, b2v)

        if KSTAGE < 5:
            nc.sync.dma_start(out=out_d[s, 0:128, :, :].rearrange("c h w -> c (h w)"),
                              in_=h1m[:, 0:NPIX])
            continue
        # ---------------- stage G: conv2 ----------------
        ph2 = sm2.tile([128, NT + 1], f32, name=f"ph2_{s}", tag="ph2_")
        for t in range(NT):
            ps = pbig.tile([128, TW], f32, name=f"ps_c2_{s}_{t}", tag="pbig")
            first = True
            for dy in range(3):
                for dx in range(3):
                    nc.tensor.matmul(
                        ps[:, :], w2T[:, 3 * dy + dx, :],
                        h1m3[:, RT * t + dy:RT * t + dy + RT, dx:dx + 56],
                        start=first, stop=(dy == 2 and dx == 2))
                    first = False
            h2r = mid.tile([128, TW], f32, name=f"h2r{s}_{t}", tag="hr")
            nc.scalar.activation(out=h2r, in_=ps[:, :], func=AF.Relu,
                                 bias=b2c[:, :], scale=cm2[:, :])
            nc.vector.scalar_tensor_tensor(
                out=h2m[:, TW * t:TW * (t + 1)],
                in0=signmask[:, TW * t:TW * (t + 1)],
                scalar=1.0,
                in1=h2r[:, :],
                op0=OP.mult, op1=OP.mult,
                accum_out=ph2[:, t:t + 1])
        p2s = sm2.tile([128, 1], f32, name=f"p2s{s}", tag="p2s")
        nc.vector.tensor_reduce(out=p2s, in_=ph2[:, 0:NT], axis=AX.X, op=OP.add)

        if KSTAGE < 6:
            nc.sync.dma_start(out=out_d[s, 0:128, :, :].rearrange("c h w -> c (h w)"),
                              in_=h2m[:, :])
            continue
        # ---------------- stage H: cm3 + coord-attention vectors ----------
        cm3 = sm2.tile([128, MC], f32, name=f"cm3_{s}", tag="cm3_")
        for mc in range(MC):
            pl3 = pvec.tile([128, 1], f32, name=f"pl3{s}_{mc}", tag="pvec")
            nc.tensor.matmul(pl3[:, :], cm3w[:, mc, :], p2s[:, :], start=True, stop=True)
            nc.vector.tensor_scalar(out=cm3[:, mc:mc + 1], in0=pl3[:, :],
                                    scalar1=cm3nb[:, mc:mc + 1], scalar2=None,
                                    op0=OP.is_gt)
        xh_pre = sm2.tile([128, W], f32, name=f"xh_pre{s}", tag="xh_pre")
        xw_pre = sm2.tile([128, W], f32, name=f"xw_pre{s}", tag="xw_pre")
        nc.vector.tensor_reduce(out=xh_pre, in_=h2m.rearrange("p (y x) -> p y x", y=H),
                                axis=AX.X, op=OP.add)
        nc.vector.tensor_reduce(out=xw_pre, in_=h2m.rearrange("p (y x) -> p x y", y=H),
                                axis=AX.X, op=OP.add)
        xcat = sm2.tile([128, KC, 2 * W], f32, name=f"xcat{s}", tag="xcat")
        for mc in range(MC):
            pxh = pvec.tile([128, W], f32, name=f"pxh{s}_{mc}", tag="pvec")
            nc.tensor.matmul(pxh[:, :], w3T[:, mc, :], xh_pre[:, :], start=True,
                             stop=B3ZERO)
            if not B3ZERO:
                nc.tensor.matmul(pxh[:, :], b3r[0:1, 128 * mc:128 * (mc + 1)],
                                 sy_row[:, :], start=False, stop=True)
            nc.scalar.activation(out=xcat[:, mc, 0:W], in_=pxh[:, :], func=AF.Copy,
                                 scale=cm3[:, mc:mc + 1])
            pxw = pvec.tile([128, W], f32, name=f"pxw{s}_{mc}", tag="pvec")
            nc.tensor.matmul(pxw[:, :], w3T[:, mc, :], xw_pre[:, :], start=True,
                             stop=B3ZERO)
            if not B3ZERO:
                nc.tensor.matmul(pxw[:, :], b3r[0:1, 128 * mc:128 * (mc + 1)],
                                 sx_row[:, :], start=False, stop=True)
            nc.scalar.activation(out=xcat[:, mc, W:2 * W], in_=pxw[:, :], func=AF.Copy,
                                 scale=cm3[:, mc:mc + 1])
        py1 = pvec.tile([MIP, 2 * W], f32, name=f"py1{s}", tag="pvec")
        for k in range(KC):
            nc.tensor.matmul(py1[:, :], caw1[:, k, :], xcat[:, k, :],
                             start=(k == 0), stop=(k == KC - 1))
        r6 = sm2.tile([MIP, 2 * W], f32, name=f"r6_{s}", tag="r6_")
        nc.scalar.activation(out=r6, in_=py1[:, :], func=AF.Relu, bias=cab1p3[:, :])
        r6b = sm2.tile([MIP, 2 * W], f32, name=f"r6b{s}", tag="r6b")
        nc.vector.tensor_scalar(out=r6b, in0=r6, scalar1=6.0, scalar2=1.0 / 6.0,
                                op0=OP.min, op1=OP.mult)
        y1 = sm2.tile([MIP, 2 * W], f32, name=f"y1_{s}", tag="y1_")
        nc.vector.tensor_tensor(out=y1, in0=r6b, in1=py1[:, :], op=OP.mult)
        ahc = sm2.tile([128, MC, W], f32, name=f"ahc{s}", tag="ahc")
        awt = sm2.tile([128, MC, W], f32, name=f"awt{s}", tag="awt")
        for mc in range(MC):
            pah = pvec.tile([128, W], f32, name=f"pah{s}_{mc}", tag="pvec")
            nc.tensor.matmul(pah[:, :], cawh[:, mc, :], y1[:, 0:W], start=True, stop=True)
            aht = sm2.tile([128, W], f32, name=f"aht{s}_{mc}", tag="aht")
            nc.scalar.activation(out=aht, in_=pah[:, :], func=AF.Sigmoid,
                                 bias=cabh[:, mc:mc + 1])
            nc.vector.tensor_scalar(out=ahc[:, mc, :], in0=aht, scalar1=cm3[:, mc:mc + 1],
                                    scalar2=None, op0=OP.mult)
            paw = pvec.tile([128, W], f32, name=f"paw{s}_{mc}", tag="pvec")
            nc.tensor.matmul(paw[:, :], caww[:, mc, :], y1[:, W:2 * W], start=True, stop=True)
            nc.scalar.activation(out=awt[:, mc, :], in_=paw[:, :], func=AF.Sigmoid,
                                 bias=cabw[:, mc:mc + 1])

        if KSTAGE < 7:
            nc.sync.dma_start(out=out_d[s, 0:128, :, :].rearrange("c h w -> c (h w)"),
                              in_=h2m[:, :])
            continue
        # ------- stage I: conv3 + CA apply + residual + relu + store -------
        for mc in range(MC):
            for pair in range(NIT // 2):
                osb = outp.tile([128, 2 * IW], f32, name=f"osb{s}_{mc}_{pair}", tag="osb")
                for hp in range(2):
                    ht = 2 * pair + hp
                    pt = pc3.tile([128, 1024], f32, name=f"pt{s}_{mc}_{ht}", tag="pc3")
                    for hh in range(2):
                        lo = IW * ht + IH * hh
                        nc.tensor.matmul(pt[:, 512 * hh:512 * hh + IH], w3T[:, mc, :],
                                         h2m[:, lo:lo + IH], start=True, stop=B3ZERO)
                        if not B3ZERO:
                            nc.tensor.matmul(pt[:, 512 * hh:512 * hh + IH],
                                             b3r[0:1, 128 * mc:128 * (mc + 1)],
                                             signmask[0:1, lo:lo + IH],
                                             start=False, stop=True)
                    ptv = pt[:, 0:1024].rearrange("p (h z) -> p h z", h=2)[:, :, 0:IH] \
                        .rearrange("p h (y x) -> p h y x", y=IR)
                    ahs = ahc[:, mc, 2 * IR * ht:2 * IR * (ht + 1)] \
                        .rearrange("p (h y) -> p h y", h=2).unsqueeze(3) \
                        .broadcast_to([128, 2, IR, W])
                    aws = awt[:, mc, :].unsqueeze(1).unsqueeze(1) \
                        .broadcast_to([128, 2, IR, W])
                    ut = mid.tile([128, IW], f32, name=f"ut{s}_{mc}_{ht}", tag="ut")
                    utv = ut.rearrange("p (h y x) -> p h y x", h=2, y=IR)
                    nc.vector.tensor_tensor(out=utv, in0=ptv, in1=ahs, op=OP.mult)
                    nc.vector.tensor_tensor(out=ptv, in0=utv, in1=aws, op=OP.mult)
                    for hh in range(2):
                        lo = IW * ht + IH * hh
                        nc.tensor.matmul(pt[:, 512 * hh:512 * hh + IH], idn[:, :],
                                         xk[mc][:, lo:lo + IH], start=False, stop=True,
                                         skip_group_check=True)
                    nc.scalar.activation(
                        out=osb[:, IW * hp:IW * hp + IW].rearrange(
                            "p (h y x) -> p h y x", h=2, y=IR),
                        in_=ptv, func=AF.Relu)
                nc.sync.dma_start(
                    out=out_d[s, 128 * mc:128 * (mc + 1), :, :].rearrange(
                        "c h w -> c (h w)")[:, 2 * IW * pair:2 * IW * (pair + 1)],
                    in_=osb[:, :])


def _build():
    nc = bacc.Bacc("TRN2", target_bir_lowering=False, debug=False)
    d = {}
    shapes = {
        "x": [BS, 512, H, W],
        "w1T": [128, KC, 128], "w2T": [128, 9, 128], "w3T": [128, MC, 128],
        "smw": [128, KC, 128], "cm1w": [128, KC, 128], "cm2w": [128, 128],
        "cm3w": [128, MC, 128], "caw1": [128, KC, MIP],
        "cawh": [MIP, MC, 128], "caww": [MIP, MC, 128],
        "idn": [128, 128], "b1v": [128, 1], "b2v": [128, 1], "b3r": [1, 512],
        "smbneg": [128, 1], "cm1nb": [128, 1], "cm2nb": [128, 1], "cm3nb": [128, MC],
        "cab1p3": [MIP, 1], "cabh": [128, MC], "cabw": [128, MC],
    }
    for name, shp in shapes.items():
        d[name] = nc.dram_tensor(name, shp, f32, kind="ExternalInput")
    d["out"] = nc.dram_tensor("out", [BS, 512, H, W], f32, kind="ExternalOutput")
    with tile.TileContext(nc) as tc, ExitStack() as ctx:
        _emit(nc, tc, ctx, d)
    nc.compile()
    return nc


def _prep_weights(i):
    """Host-side rearrangement of the full (replicated) weights."""
    N = NPIX
    w1 = i["w1"][:, :, 0, 0]                     # [128, 512]
    w3 = i["w3"][:, :, 0, 0]                     # [512, 128]
    caw1 = i["ca_w1"][:, :, 0, 0]                # [16, 512]
    cawh = i["ca_wh"][:, :, 0, 0]                # [512, 16]
    caww = i["ca_ww"][:, :, 0, 0]                # [512, 16]
    smv = i["sm_w"][0, :, 0, 0]                  # [512]
    f = np.float32
    return {
        "w1T": np.ascontiguousarray(w1.T.reshape(KC, 128, 128).transpose(1, 0, 2), f),
        "w2T": np.ascontiguousarray(i["w2"].transpose(1, 2, 3, 0).reshape(128, 9, 128), f),
        "w3T": np.ascontiguousarray(w3.T.reshape(128, MC, 128), f),
        "smw": np.ascontiguousarray(
            np.broadcast_to(smv.reshape(KC, 128).T[:, :, None], (128, KC, 128)), f),
        "cm1w": np.ascontiguousarray(
            (i["cm1_w"] / N).T.reshape(KC, 128, 128).transpose(1, 0, 2), f),
        "cm2w": np.ascontiguousarray((i["cm2_w"] / N).T, f),
        "cm3w": np.ascontiguousarray((i["cm3_w"] / N).T.reshape(128, MC, 128), f),
        "caw1": np.ascontiguousarray(
            (caw1 / W).T.reshape(KC, 128, MIP).transpose(1, 0, 2), f),
        "cawh": np.ascontiguousarray(cawh.T.reshape(MIP, MC, 128), f),
        "caww": np.ascontiguousarray(caww.T.reshape(MIP, MC, 128), f),
        "idn": np.eye(128, dtype=f),
        "b1v": i["b1"].reshape(128, 1).astype(f),
        "b2v": i["b2"].reshape(128, 1).astype(f),
        "b3r": i["b3"].reshape(1, 512).astype(f),
        "smbneg": np.full((128, 1), -i["sm_b"][0], f),
        "cm1nb": (-i["cm1_b"]).reshape(128, 1).astype(f),
        "cm2nb": (-i["cm2_b"]).reshape(128, 1).astype(f),
        "cm3nb": np.ascontiguousarray((-i["cm3_b"]).reshape(MC, 128).T, f),
        "cab1p3": (i["ca_b1"] + 3.0).reshape(MIP, 1).astype(f),
        "cabh": np.ascontiguousarray(i["ca_bh"].reshape(MC, 128).T, f),
        "cabw": np.ascontiguousarray(i["ca_bw"].reshape(MC, 128).T, f),
    }


_NC_CACHE = {}


def _get_nc():
    key = (B3ZERO, KREPEAT)
    if key not in _NC_CACHE:
        _NC_CACHE[key] = _build()
    return _NC_CACHE[key]


def kernel(**inputs):
    global B3ZERO
    B3ZERO = not np.any(inputs["b3"])
    nc = _get_nc()
    wmap = _prep_weights(inputs)
    x = np.ascontiguousarray(inputs["x"], np.float32)
    in_maps = []
    for c in range(NCORES):
        m = dict(wmap)
        m["x"] = np.ascontiguousarray(x[BS * c:BS * (c + 1)])
        in_maps.append(m)
    res = run_bass_kernel_spmd(nc, in_maps, core_ids=list(range(NCORES)))
    return np.concatenate([r["out"] for r in res.results], axis=0)
